# revision 52
# baseline (speedup 1.0000x reference)
"""Trainium2 Bass kernel for causal multi-head attention with RoPE.

Problem: x[2,2048,2048] -> q/k/v projections (+bias), RoPE(q,k), causal SDPA
(16 heads, hd=128), output projection (+bias).

Sharding: tensor-parallel over heads. Each of the 8 cores computes 2 heads:
its slice of the q/k/v projections, attention for its heads, and a partial
output projection (contraction over its 256 local dims). The host sums the 8
partial outputs and adds the (exactly foldable) bv/bo bias terms.

Single fully-interleaved pipeline (one pass, PE ~97% busy in the cost
model; 338.5us -> 289.7us vs the phase-sequential baseline):

  Each 512-token chunk runs the Q^T/K^T/V projections as four sequential
  2-bank PSUM passes (q, k, v01, v23; 4 banks total with double buffering)
  with RoPE fused on ACT+DVE, leaving 4 PSUM banks for attention, whose
  work is emitted as small "pieces" (one k-tile pair of scores+exp+mask+
  denominator-add, or one AV pair) zipped evenly between projection
  di-steps: the ACT exp stream (~610ns/pair) is slower than its own PE
  work (~430ns/pair), so attention alone can never keep PE busy —
  projection matmuls between pieces absorb the exp latency. Batch-0
  attention groups run during chunks 1-4, batch-1 groups during chunks
  5-7, and the last slot + all 32 output-projection blocks after chunk 7
  (block 0 on the drained attention PSUM bank to cover chunk 7's drain;
  PE warm-up matmuls cover the initial DMA latency; chunk-0 loads are
  split to match di-consumption order; the last blocks' y DMAs are
  half-split to shorten the terminal HWDGE/DMA/sem chain).

  Attention is flash-style per (batch, 256-token q-group, head): scores^T
  k-tile pairs [128k x 256q] on PE -> exp on ACT (constant -2 bias cancels
  in normalization; no running max needed, |scaled scores| < 9) -> causal
  mask multiply + per-tile denominator accumulation on DVE (fp16, 2x mode)
  -> AV accumulation in PSUM -> unnormalized-output staging copy on ACT
  (recycles the PSUM bank fast) -> cross-partition denominator sum via
  gpsimd.partition_all_reduce (Pool engine) -> DVE reciprocal + normalize.
  This keeps the softmax denominator entirely off the tensor engine (the
  baseline's ones-matmuls were ~10% of PE time); cos/sin tables are fp16
  (halves RoPE DVE cost via the 2x mode). The last k-tile of each group
  is computed narrow (its lower 128 query columns are fully masked), with
  one flat 384-column exp per diagonal pair; PE cost scales with output
  free-size only, so this is the minimal rectangle cover of the causal
  triangle at 128-wide k granularity.
"""

import numpy as np

import concourse.bacc as bacc
import concourse.bass_isa as bass_isa
import concourse.mybir as mybir
import concourse.tile as tile
from concourse.bass_utils import run_bass_kernel_spmd

# problem constants (fixed by the graded problem)
B, S, D, H, HD = 2, 2048, 2048, 16, 128
T = B * S            # 4096 tokens
P = 128              # partitions
NCORES = 8
HPC = H // NCORES    # 2 heads per core
DL = HPC * HD        # 256 local projection dims per core
DIN = D // P         # 16 contraction blocks
CH = 512             # token chunk for the projection phase
NCH = T // CH        # 8
QG = 256             # q-group width in attention
NG = S // QG         # 8 q-groups per (batch, head)
NTB = T // P         # 32 token blocks for the output projection
SCALE = 1.0 / float(np.sqrt(HD))
EXP_BIAS = -2.0      # constant exp bias; cancels in normalization

f32 = mybir.dt.float32
f16 = mybir.dt.float16
AF = mybir.ActivationFunctionType
ADD = bass_isa.ReduceOp.add


def _build(repeat=1):
    nc = bacc.Bacc("TRN2", target_bir_lowering=False, debug=False)

    xp_d = nc.dram_tensor("xp", [P, DIN, T], f16, kind="ExternalInput")
    wq_d = nc.dram_tensor("wqt", [P, DIN, DL], f16, kind="ExternalInput")
    wk_d = nc.dram_tensor("wkt", [P, DIN, DL], f16, kind="ExternalInput")
    wv_d = nc.dram_tensor("wvt", [P, DIN, DL], f16, kind="ExternalInput")
    wo_d = nc.dram_tensor("wot", [P, HPC, D], f16, kind="ExternalInput")
    c2_d = nc.dram_tensor("c2", [P, T], f16, kind="ExternalInput")
    s2_d = nc.dram_tensor("s2", [P, T], f16, kind="ExternalInput")
    msk_d = nc.dram_tensor("msk", [P, 896], f16, kind="ExternalInput")
    bq_d = nc.dram_tensor("bq2", [P, HPC], f32, kind="ExternalInput")
    bk_d = nc.dram_tensor("bk2", [P, HPC], f32, kind="ExternalInput")
    eb_d = nc.dram_tensor("ebias", [P, 1], f32, kind="ExternalInput")
    y_d = nc.dram_tensor("y", [T, D], f16, kind="ExternalOutput")

    with tile.TileContext(nc) as tc:
      for _rep in range(repeat):
        with tc.tile_pool(name="persist", bufs=1) as pp:
            qt = pp.tile([P, HPC, T], f16, tag="qt")
            kt = pp.tile([P, HPC, T], f16, tag="kt")
            vt = pp.tile([P, NTB, DL], f16, tag="vt")
            ao = pp.tile([P, HPC, T], f16, tag="ao")
            wo = pp.tile([P, HPC, D], f16, tag="wo")
            bq = pp.tile([P, HPC], f32, tag="bq")
            bk = pp.tile([P, HPC], f32, tag="bk")
            msk = pp.tile([P, 896], f16, tag="msk")
            ebias = pp.tile([P, 1], f32, tag="ebias")

            with tc.tile_pool(name="probsp", bufs=4) as prp, \
                 tc.tile_pool(name="normp", bufs=5) as nrp, \
                 tc.tile_pool(name="attps", bufs=2, space="PSUM") as aps, \
                 tc.tile_pool(name="attpso", bufs=2, space="PSUM") as apo, \
                 tc.tile_pool(name="yp", bufs=4) as yp:

                # ---------- attention piece lists ----------
                # Attention is emitted as small "pieces" (one k-tile pair of
                # scores+exp, or one AV pair) that the chunk loop zips
                # between projection di-steps: the ACT exp stream (~610ns
                # per pair) is slower than its PE work (~430ns), so PE-side
                # projection work must sit between attention pieces.
                def att_unit_pieces(b, g, h):
                    """Piece closures for one (batch, q-group, head) unit:
                    S0 S1 A0 S2 A1 ... Sg A(g-2) A(g-1) A(g)+finish."""
                    nk = (g + 1) * 2          # 128-wide k tiles
                    q0 = b * S + g * QG
                    # the last k-tile's lower 128 queries are fully masked:
                    # compute it narrow (upper 128 q only). g=0 keeps the
                    # full path so PSUM start/stop stays column-uniform.
                    narrow = g >= 1
                    st = {}

                    def s_piece(pi):
                        if pi == 0:
                            st["probs"] = prp.tile(
                                [P, NG * 2, QG], f16, tag="probs",
                                name=f"pr{_rep}_{b}_{g}_{h}")
                            st["tsum"] = nrp.tile(
                                [P, QG], f16, tag="tsum",
                                name=f"ts{_rep}_{b}_{g}_{h}")
                        probs, tsum = st["probs"], st["tsum"]
                        ps_s = aps.tile([P, 2, QG], f32, tag="pss",
                                        name=f"pss{_rep}_{b}_{g}_{h}_{pi}")
                        if pi == g and narrow:
                            k0 = b * S + 2 * g * P
                            nc.tensor.matmul(ps_s[:, 0],
                                             kt[:, h, k0:k0 + P],
                                             qt[:, h, q0:q0 + QG],
                                             start=True, stop=True)
                            nc.tensor.matmul(ps_s[:, 1, 0:P],
                                             kt[:, h, k0 + P:k0 + 2 * P],
                                             qt[:, h, q0 + P:q0 + QG],
                                             start=True, stop=True)
                            # one flat exp over the 384 live columns
                            pf = probs[:, 2 * pi:2 * pi + 2].rearrange(
                                "p a b -> p (a b)")
                            sf = ps_s[:].rearrange("p a b -> p (a b)")
                            nc.scalar.activation(pf[:, 0:QG + P],
                                                 sf[:, 0:QG + P], AF.Exp,
                                                 bias=ebias[:, 0:1],
                                                 scale=SCALE)
                            nc.vector.tensor_mul(probs[:, 2 * pi],
                                                 probs[:, 2 * pi],
                                                 msk[:, 384:640])
                            nc.vector.tensor_mul(probs[:, 2 * pi + 1, 0:P],
                                                 probs[:, 2 * pi + 1, 0:P],
                                                 msk[:, 384:512])
                            nc.vector.tensor_add(tsum[:], tsum[:],
                                                 probs[:, 2 * pi])
                            nc.vector.tensor_add(tsum[:, P:QG],
                                                 tsum[:, P:QG],
                                                 probs[:, 2 * pi + 1, 0:P])
                            return
                        for i in range(2):
                            t_ = 2 * pi + i
                            k0 = b * S + t_ * P
                            nc.tensor.matmul(ps_s[:, i], kt[:, h, k0:k0 + P],
                                             qt[:, h, q0:q0 + QG],
                                             start=True, stop=True)
                        nc.scalar.activation(probs[:, 2 * pi:2 * pi + 2],
                                             ps_s[:], AF.Exp,
                                             bias=ebias[:, 0:1], scale=SCALE)
                        for i in range(2):
                            t_ = 2 * pi + i
                            off = t_ * P - g * QG
                            if off >= 0:   # diagonal tile: causal mask
                                nc.vector.tensor_mul(
                                    probs[:, t_], probs[:, t_],
                                    msk[:, 384 - off:640 - off])
                        if pi == 0:
                            nc.vector.tensor_add(tsum[:], probs[:, 0],
                                                 probs[:, 1])
                        else:
                            nc.vector.tensor_add(tsum[:], tsum[:],
                                                 probs[:, 2 * pi])
                            nc.vector.tensor_add(tsum[:], tsum[:],
                                                 probs[:, 2 * pi + 1])

                    def a_piece(pi):
                        probs = st["probs"]
                        if pi == 0:
                            st["pso"] = apo.tile([P, QG], f32, tag="pso",
                                                 name=f"pso{_rep}_{b}_{g}_{h}")
                        ps_o = st["pso"]
                        if pi == g and narrow:
                            # narrow tile first (no stop), then the full
                            # diag tile with stop=True so every column's
                            # accumulation epoch closes on the last matmul
                            hs = h * P
                            blk = b * (S // P) + 2 * g + 1
                            nc.tensor.matmul(ps_o[:, P:QG],
                                             vt[:, blk, hs:hs + P],
                                             probs[:, 2 * g + 1, 0:P],
                                             start=False, stop=False)
                            blk = b * (S // P) + 2 * g
                            nc.tensor.matmul(ps_o[:],
                                             vt[:, blk, hs:hs + P],
                                             probs[:, 2 * g],
                                             start=False, stop=True)
                        else:
                            for i in range(2):
                                t_ = 2 * pi + i
                                blk = b * (S // P) + t_
                                nc.tensor.matmul(ps_o[:],
                                                 vt[:, blk,
                                                    h * P:(h + 1) * P],
                                                 probs[:, t_],
                                                 start=(t_ == 0),
                                                 stop=(t_ == nk - 1))
                        if pi == g:
                            # stage unnormalized out to SBUF (frees PSUM
                            # fast); all_reduce/recip/mul run off-path
                            aou = nrp.tile([P, QG], f16, tag="aou",
                                           name=f"au{_rep}_{b}_{g}_{h}")
                            nc.scalar.copy(aou[:], ps_o[:])
                            d = nrp.tile([P, QG], f32, tag="dsum",
                                         name=f"d{_rep}_{b}_{g}_{h}")
                            nc.gpsimd.partition_all_reduce(
                                d[:], st["tsum"][:], P, ADD)
                            r = nrp.tile([P, QG], f32, tag="rsum",
                                         name=f"r{_rep}_{b}_{g}_{h}")
                            nc.vector.reciprocal(r[:], d[:])
                            nc.vector.tensor_mul(ao[:, h, q0:q0 + QG],
                                                 aou[:], r[:])

                    pieces = []
                    for pi in range(g + 1):
                        pieces.append(lambda pi=pi: s_piece(pi))
                        if pi >= 2:
                            pieces.append(lambda pi=pi: a_piece(pi - 2))
                    for pi in range(max(g - 1, 0), g + 1):
                        pieces.append(lambda pi=pi: a_piece(pi))
                    return pieces

                def att_slot_pieces(b, g2):
                    pieces = []
                    for g in (g2, g2 + 1):
                        for h in range(HPC):
                            pieces.extend(att_unit_pieces(b, g, h))
                    return pieces

                class Zipper:
                    """Spreads a piece list evenly across backbone steps."""
                    def __init__(self, pieces, nsteps):
                        self.pieces = pieces
                        self.nsteps = max(nsteps, 1)
                        self.j = 0
                        self.step = 0

                    def tick(self):
                        self.step += 1
                        n = len(self.pieces)
                        while (self.j < n and
                               self.j * self.nsteps <= self.step * n):
                            self.pieces[self.j]()
                            self.j += 1

                    def drain(self):
                        while self.j < len(self.pieces):
                            self.pieces[self.j]()
                            self.j += 1

                def outproj(tb, yps, zp=None, split_dma=False, tag="psy"):
                    """Partial out-projection for one 128-token block."""
                    y_sb = yp.tile([P, D], f16, tag="ysb",
                                   name=f"ysb{_rep}_{tb}")
                    for dc in range(D // 512):
                        ps_y = yps.tile([P, 512], f32, tag=tag,
                                        name=f"psy{_rep}_{tb}_{dc}")
                        for hf in range(HPC):
                            nc.tensor.matmul(
                                ps_y[:], ao[:, hf, tb * P:(tb + 1) * P],
                                wo[:, hf, dc * 512:(dc + 1) * 512],
                                start=(hf == 0), stop=(hf == HPC - 1))
                        if zp is not None:
                            zp.tick()
                        if (tb + dc) % 2 == 0:
                            nc.scalar.copy(y_sb[:, dc * 512:(dc + 1) * 512],
                                           ps_y[:])
                        else:
                            nc.vector.tensor_copy(
                                y_sb[:, dc * 512:(dc + 1) * 512], ps_y[:])
                    if split_dma == "fine":
                        # 3/4 + 1/4 split: only a 512-column transfer sits
                        # after the last copy
                        nc.sync.dma_start(
                            y_d.ap()[tb * P:(tb + 1) * P, 0:1536],
                            y_sb[:, 0:1536])
                        nc.sync.dma_start(
                            y_d.ap()[tb * P:(tb + 1) * P, 1536:2048],
                            y_sb[:, 1536:2048])
                    elif split_dma:
                        # half DMAs pipeline behind the dc1/dc3 copies so
                        # only a 1024-column transfer sits after the tail
                        for half in range(2):
                            nc.sync.dma_start(
                                y_d.ap()[tb * P:(tb + 1) * P,
                                         half * 1024:(half + 1) * 1024],
                                y_sb[:, half * 1024:(half + 1) * 1024])
                    else:
                        nc.sync.dma_start(y_d.ap()[tb * P:(tb + 1) * P, :],
                                          y_sb[:])

                # ---------- projection chunks with interleaved slots ----------
                with tc.tile_pool(name="wpool", bufs=1) as wp, \
                     tc.tile_pool(name="xpool", bufs=2) as xp_pool, \
                     tc.tile_pool(name="cspool", bufs=2) as csp, \
                     tc.tile_pool(name="sbqpool", bufs=4) as sbqp, \
                     tc.tile_pool(name="projps", bufs=2, space="PSUM") as pps:
                    wq = wp.tile([P, DIN, DL], f16, tag="wq")
                    wk = wp.tile([P, DIN, DL], f16, tag="wk")
                    wv = wp.tile([P, DIN, DL], f16, tag="wv")
                    HDIN = DIN // 2
                    HH = P // 2
                    for ch in range(NCH):
                        t0 = ch * CH
                        xh = [xp_pool.tile([P, HDIN, CH], f16, tag=f"xh{i}",
                                           name=f"xh{_rep}_{ch}_{i}")
                              for i in range(2)]
                        c2f = csp.tile([P, CH], f16, tag="c2c",
                                       name=f"c2c{_rep}_{ch}")
                        s2f = csp.tile([P, CH], f16, tag="s2c",
                                       name=f"s2c{_rep}_{ch}")
                        if ch == 0:
                            # PE warm-up matmuls bridge the ~3.3us
                            # first-DMA latency and finish the p-state
                            # ramp before real work arrives
                            # (reads msk before its DMA: garbage values into
                            # a never-read psum; the WAR dep only delays the
                            # msk load, which isn't needed until chunk 1)
                            ps_w = aps.tile([P, 2, QG], f32, tag="pss",
                                            name=f"warmps{_rep}")
                            for _ in range(14):
                                nc.tensor.matmul(ps_w[:, 0], msk[:, 0:P],
                                                 msk[:, 256:256 + QG],
                                                 start=True, stop=True)
                            # delivery ordered to match consumption: the
                            # q-pass streams (wq_di, x_di) pairs, the k-pass
                            # needs wk ~7us in, RoPE needs c2/s2/bq/bk ~8us
                            # in, the v-passes need wv ~14us in
                            nc.sync.dma_start(wq[:, 0:2], wq_d.ap()[:, 0:2])
                            nc.sync.dma_start(xh[0][:, 0:2],
                                              xp_d.ap()[:, 0:2, t0:t0 + CH])
                            nc.sync.dma_start(wq[:, 2:8], wq_d.ap()[:, 2:8])
                            nc.sync.dma_start(xh[0][:, 2:4],
                                              xp_d.ap()[:, 2:4, t0:t0 + CH])
                            nc.sync.dma_start(xh[0][:, 4:6],
                                              xp_d.ap()[:, 4:6, t0:t0 + CH])
                            nc.sync.dma_start(xh[0][:, 6:8],
                                              xp_d.ap()[:, 6:8, t0:t0 + CH])
                            nc.sync.dma_start(wq[:, 8:12], wq_d.ap()[:, 8:12])
                            nc.sync.dma_start(wq[:, 12:16],
                                              wq_d.ap()[:, 12:16])
                            nc.sync.dma_start(xh[1][:, 0:2],
                                              xp_d.ap()[:, 8:10, t0:t0 + CH])
                            nc.sync.dma_start(xh[1][:, 2:4],
                                              xp_d.ap()[:, 10:12, t0:t0 + CH])
                            nc.sync.dma_start(xh[1][:, 4:6],
                                              xp_d.ap()[:, 12:14, t0:t0 + CH])
                            nc.sync.dma_start(xh[1][:, 6:8],
                                              xp_d.ap()[:, 14:16, t0:t0 + CH])
                            nc.sync.dma_start(wk[:, 0:4], wk_d.ap()[:, 0:4])
                            nc.sync.dma_start(wk[:, 4:8], wk_d.ap()[:, 4:8])
                            nc.sync.dma_start(bq[:], bq_d.ap())
                            nc.sync.dma_start(bk[:], bk_d.ap())
                            nc.sync.dma_start(ebias[:], eb_d.ap())
                            nc.sync.dma_start(c2f[:],
                                              c2_d.ap()[:, t0:t0 + CH])
                            nc.sync.dma_start(s2f[:],
                                              s2_d.ap()[:, t0:t0 + CH])
                            nc.sync.dma_start(wk[:, 8:12], wk_d.ap()[:, 8:12])
                            nc.sync.dma_start(wk[:, 12:16], wk_d.ap()[:, 12:16])
                            nc.sync.dma_start(wv[:, 0:8], wv_d.ap()[:, 0:8])
                            nc.sync.dma_start(wv[:, 8:16], wv_d.ap()[:, 8:16])
                            nc.sync.dma_start(msk[:], msk_d.ap())
                        else:
                            for hf in range(2):
                                nc.sync.dma_start(
                                    xh[hf][:],
                                    xp_d.ap()[:, hf * HDIN:(hf + 1) * HDIN,
                                              t0:t0 + CH])
                            nc.sync.dma_start(c2f[:],
                                              c2_d.ap()[:, t0:t0 + CH])
                            nc.sync.dma_start(s2f[:],
                                              s2_d.ap()[:, t0:t0 + CH])
                        if ch == 1:
                            nc.sync.dma_start(wo[:], wo_d.ap())

                        def xt(di):
                            hf, dl = divmod(di, HDIN)
                            return xh[hf][:, dl]          # [P, CH]

                        # attention zipped between this chunk's di-steps
                        # (its k-range was projected by previous chunks)
                        if 1 <= ch <= 4:
                            zp = Zipper(att_slot_pieces(0, 2 * (ch - 1)), 64)
                        elif ch >= 5:
                            zp = Zipper(att_slot_pieces(1, 2 * (ch - 5)), 64)
                        else:
                            zp = Zipper([], 64)

                        # pass 1/2: Q^T then K^T ([dim, token]), fused RoPE
                        for name, wmat, bias_t, dst in (
                                ("q", wq, bq, qt), ("k", wk, bk, kt)):
                            ps_m = [pps.tile([P, CH], f32, tag=f"pp{m}",
                                             name=f"pp{name}{_rep}_{ch}_{m}")
                                    for m in range(2)]
                            for di in range(DIN):
                                st = (di == 0)
                                sp = (di == DIN - 1)
                                for m in range(2):
                                    nc.tensor.matmul(
                                        ps_m[m][:],
                                        wmat[:, di, m * P:(m + 1) * P],
                                        xt(di), start=st, stop=sp)
                                zp.tick()
                            # RoPE: rot = (q+b)*C2 + halfswap(q+b)*S2 (S2's
                            # top half carries the negative sign, host-built)
                            for m in range(2):
                                sbq = sbqp.tile([P, CH], f16, tag="sbq")
                                nc.scalar.activation(sbq[:], ps_m[m][:],
                                                     AF.Identity,
                                                     bias=bias_t[:, m:m + 1])
                                sw = sbqp.tile([P, CH], f16, tag="sw")
                                nc.vector.tensor_copy(sw[0:HH, :], sbq[HH:P, :])
                                nc.vector.tensor_copy(sw[HH:P, :], sbq[0:HH, :])
                                dslc = dst[:, m, t0:t0 + CH]
                                nc.vector.tensor_mul(dslc, sbq[:], c2f[:])
                                nc.vector.tensor_mul(sw[:], sw[:], s2f[:])
                                nc.vector.tensor_add(dslc, dslc, sw[:])

                        # passes 3/4: V in [token, dim] layout, 2 blocks each
                        for vp in range(2):
                            ps_m = [pps.tile([P, CH], f32, tag=f"pp{m}",
                                             name=f"ppv{_rep}_{ch}_{vp}_{m}")
                                    for m in range(2)]
                            for di in range(DIN):
                                st = (di == 0)
                                sp = (di == DIN - 1)
                                for m in range(2):
                                    s_ = 2 * vp + m
                                    nc.tensor.matmul(
                                        ps_m[m][:, 0:DL],
                                        xt(di)[:, s_ * P:(s_ + 1) * P],
                                        wv[:, di], start=st, stop=sp)
                                zp.tick()
                            for m in range(2):
                                blk = t0 // P + 2 * vp + m
                                nc.scalar.copy(vt[:, blk, :], ps_m[m][:, 0:DL])

                        zp.drain()

                # projection pools closed: 4 PSUM banks free.
                # The last attention slot (b1 g6/g7 — whose QUERY tokens
                # come from chunk 7, so every piece waits its RoPE drain)
                # is zipped into the out-projection backbone; block 0 runs
                # its psums on the already-drained attention pso bank so PE
                # has work while chunk 7's projection psums drain.
                zp = Zipper(att_slot_pieces(1, 6), 72)
                outproj(0, apo, tag="pso")
                with tc.tile_pool(name="yps2", bufs=4, space="PSUM") as yps:
                    for tb in range(1, NTB):
                        outproj(tb, yps, zp, split_dma=(tb >= NTB - 5))
                    zp.drain()

    nc.compile()
    return nc


_NC = None


def _get_nc():
    global _NC
    if _NC is None:
        _NC = _build()
    return _NC


def _prep_inputs(x, wq, bq, wk, bk, wv, bv, wo, bo, freqs_cos, freqs_sin):
    """Host-side marshalling: transposes/permutations/shards. Pure numpy."""
    f = np.float32
    x = np.asarray(x, f)
    xT = x.reshape(T, D).T                                   # [D, T]
    xp = np.ascontiguousarray(
        xT.reshape(DIN, P, T).transpose(1, 0, 2)).astype(np.float16)

    # per-head row permutation: [evens, odds] so rope pairs sit in partition halves
    perm1 = np.concatenate([np.arange(0, HD, 2), np.arange(1, HD, 2)])
    perm = np.concatenate([h * HD + perm1 for h in range(HPC)])  # [DL]

    cosT = np.asarray(freqs_cos, f).T                       # [64, S]
    sinT = np.asarray(freqs_sin, f).T
    c2 = np.ascontiguousarray(
        np.tile(np.concatenate([cosT, cosT], 0), (1, B))).astype(np.float16)
    s2 = np.ascontiguousarray(
        np.tile(np.concatenate([-sinT, sinT], 0), (1, B))).astype(np.float16)

    jj, kk = np.meshgrid(np.arange(896), np.arange(P), indexing="xy")
    mskv = (jj - 384 >= kk).astype(np.float16)              # [P, 896] binary

    ebv = np.full((P, 1), EXP_BIAS, np.float32)

    def slc(w, permute):
        wc_all = []
        for c in range(NCORES):
            wc = np.asarray(w, f)[c * DL:(c + 1) * DL]      # [DL, D]
            if permute:
                wc = wc[perm]
            wt = np.ascontiguousarray(
                wc.T.reshape(DIN, P, DL).transpose(1, 0, 2))  # [P, DIN, DL]
            wc_all.append(wt.astype(np.float16))
        return wc_all

    wq_all = slc(wq, True)
    wk_all = slc(wk, True)
    wv_all = slc(wv, False)

    wo = np.asarray(wo, f)
    wo_all, bq_all, bk_all = [], [], []
    for c in range(NCORES):
        woc = wo[:, c * DL:(c + 1) * DL]                    # [D, DL]
        wot = np.ascontiguousarray(
            woc.T.reshape(HPC, P, D).transpose(1, 0, 2))    # [P, HPC, D]
        wo_all.append(wot.astype(np.float16))
        bqc = np.asarray(bq, f)[c * DL:(c + 1) * DL][perm]
        bkc = np.asarray(bk, f)[c * DL:(c + 1) * DL][perm]
        bq_all.append(np.ascontiguousarray(bqc.reshape(HPC, P).T))
        bk_all.append(np.ascontiguousarray(bkc.reshape(HPC, P).T))

    in_maps = []
    for c in range(NCORES):
        in_maps.append({
            "xp": xp, "wqt": wq_all[c], "wkt": wk_all[c], "wvt": wv_all[c],
            "wot": wo_all[c], "c2": c2, "s2": s2, "msk": mskv,
            "bq2": bq_all[c], "bk2": bk_all[c], "ebias": ebv,
        })
    return in_maps


def _run(in_maps, trace=False):
    nc = _get_nc()
    return run_bass_kernel_spmd(nc, in_maps, core_ids=list(range(NCORES)),
                                trace=trace)


def kernel(**inputs):
    in_maps = _prep_inputs(**inputs)
    res = _run(in_maps)
    y = np.zeros((T, D), np.float32)
    for c in range(NCORES):
        y += res.results[c]["y"].astype(np.float32)
    bv = np.asarray(inputs["bv"], np.float32)
    bo = np.asarray(inputs["bo"], np.float32)
    wo = np.asarray(inputs["wo"], np.float32)
    y += (bo + bv @ wo.T)[None, :]
    return y.reshape(B, S, D)
